# revision 17
# baseline (speedup 1.0000x reference)
"""LSTMCell forward on 8 Trainium2 NeuronCores (Bass/Tile, SPMD data-parallel).

Strategy:
  - Shard the batch (32768) across 8 cores: 4096 rows each. The kernel is
    Tensor-engine bound: 2*4096*1024*2048 = 17.2 GFLOP/core at 78.6 TF/s
    (1 col/cycle @ 2.4 GHz for 16-bit operands) floors the body at ~220 us.
    fp8 DoubleRow (2x PE) was evaluated and rejected: e4m3 operand
    quantization gives 9.5e-2 rel err vs the 2e-2 tolerance (measured on the
    actual inputs); fp16 gives 8e-4.
  - All IO in fp16: halves HBM traffic vs fp32 (24 MB/core, ~67 us at
    358 GB/s — fully hidden under the PE) and doubles DVE throughput.
  - Transposed-z layout: compute z^T [2048 gate dims, batch] so the gate dim
    lands on PSUM partitions. Stationary operand = W 128x128 blocks, moving
    operand = xh^T batch columns (N=512, the PSUM-bank max, so LDWEIGHTS
    hides under the 512-cycle stream). The per-gate bias is then
    per-partition, so ACT fuses PSUM-evacuation + bias + sigmoid/tanh into
    ONE pass (no DVE bias add).
  - Per output block jb (16 of them: gates i,f,o,g x 4 dh-blocks): accumulate
    8 k-chunks into one PSUM bank [128, 512] (8-bank rotation keeps the PE
    several groups ahead of the ACT drain), ACT-evacuate to SBUF fp16.
  - jb order is gate-major per dh-block (i,f,o,g of block d together) so the
    elementwise epilogue for block d (C_new = f*C + i*g; h_new = o*tanh)
    runs on DVE while the PE streams block d+1; the epilogue tanh is
    deferred past the evacuations to keep the in-order ACT queue draining
    PSUM without stalls.
  - Batch tiled in macro-chunks of 512 columns, double-buffered DMA.
  - Prologue: macro-0 inputs and the first weight block are DMA'd ahead of
    the remaining 15 weight blocks (host pre-swizzles W so each block is a
    contiguous 4 KB/partition transfer), so the PE starts ~3.5 us in instead
    of waiting for the full 4 MB weight load.
"""
import sys
from contextlib import nullcontext

if "/opt/trn_rl_repo" not in sys.path:
    sys.path.insert(0, "/opt/trn_rl_repo")

import numpy as np
import concourse.bass as bass
import concourse.mybir as mybir
from concourse.tile import TileContext
from concourse.bass_utils import run_bass_kernel_spmd

F32 = mybir.dt.float32
F16 = mybir.dt.float16
AF = mybir.ActivationFunctionType

N_CORES = 8
P = 128
DH = 512
DH4 = 4 * DH            # 2048
K = 1024                # concat(x, h) contraction dim
KT = K // P             # 8 k-chunks
NJ = DH4 // P           # 16 output column blocks of z^T
ND = DH // P            # 4 dh blocks
B_FULL = 32768
B_CORE = B_FULL // N_CORES   # 4096
BM = 512                     # batch columns per macro-chunk (= PSUM bank)
NMACRO = B_CORE // BM        # 8


def fanout_multi_waits(nc):
    """This walrus build rejects >1 sync wait per instruction: fan extra
    waits out onto single-wait NoOps on the same (in-order) engine."""
    n = 0
    for f in nc.m.functions:
        for bb in f.blocks:
            new = []
            for inst in bb.instructions:
                si = inst.sync_info
                waits = list(si.on_wait) if si and si.on_wait else []
                if len(waits) > 1:
                    for w in waits[:-1]:
                        nop = mybir.InstNoOp(name=f"waitfan_{n}", ins=[], outs=[])
                        n += 1
                        nop.engine = inst.engine
                        nop.sync_info = mybir.SyncInfo(on_wait=[w], on_update=[])
                        new.append(nop)
                    si.on_wait = [waits[-1]]
                new.append(inst)
            bb.instructions = new
    return n


def build_nc(loop_n=None, unroll=1):
    """Build the per-core program. loop_n wraps the body in a device-side
    For_i repeat (timing probe; outputs unchanged since the body is
    idempotent). unroll emits that many body copies per loop iteration —
    the For_i back edge is a cross-engine sync point that idles the PE for
    several us, so fewer iterations -> less measurement artifact."""
    nc = bass.Bass()
    xhT = nc.dram_tensor("xhT", [K, B_CORE], F16, kind="ExternalInput")
    CT = nc.dram_tensor("CT", [DH, B_CORE], F16, kind="ExternalInput")
    # W host-swizzled to [128, jb, kt*128] so each jb block is a fully
    # contiguous 4 KB-per-partition DMA.
    W = nc.dram_tensor("W", [P, NJ * K], F16, kind="ExternalInput")
    biasT = nc.dram_tensor("biasT", [P, NJ], F32, kind="ExternalInput")
    CnT = nc.dram_tensor("CnT", [DH, B_CORE], F16, kind="ExternalOutput")
    HnT = nc.dram_tensor("HnT", [DH, B_CORE], F16, kind="ExternalOutput")

    xhT_r = xhT[:].rearrange("(kt p) b -> p kt b", p=P)     # [128, 8, 4096]
    W_r = W[:].rearrange("p (j kt c) -> p j kt c", j=NJ, kt=KT)  # [128,16,8,128]
    CT_r = CT[:].rearrange("(d p) b -> p d b", p=P)         # [128, 4, 4096]
    Cn_r = CnT[:].rearrange("(d p) b -> p d b", p=P)
    Hn_r = HnT[:].rearrange("(d p) b -> p d b", p=P)

    with TileContext(nc) as tc:
        with (
            tc.tile_pool(name="const", bufs=1) as const,
            tc.tile_pool(name="io", bufs=2) as io,
            tc.tile_pool(name="work", bufs=2) as work,
            tc.tile_pool(name="psum", bufs=8, space=bass.MemorySpace.PSUM) as psum,
        ):
            # Split the weight load per column-block (jb0 first) so the first
            # matmul group only waits on ~300 KB, not the whole 4 MB.
            w_t = const.tile([P, NJ, KT, P], F16)
            bias_t = const.tile([P, NJ], F32)
            # Macro-0 inputs first: the PE can start ~3.5 us in (xh0 + W jb0)
            # while the remaining W blocks stream in behind it.
            xh0_t = const.tile([P, KT, BM], F16)
            nc.sync.dma_start(out=xh0_t[:], in_=xhT_r[:, :, 0:BM])
            nc.sync.dma_start(out=w_t[:, 0, :, :], in_=W_r[:, 0, :, :])
            nc.sync.dma_start(out=bias_t[:], in_=biasT[:])
            c0_t = const.tile([P, ND, BM], F16)
            nc.sync.dma_start(out=c0_t[:], in_=CT_r[:, :, 0:BM])
            for jb in range(1, NJ):
                nc.sync.dma_start(out=w_t[:, jb, :, :], in_=W_r[:, jb, :, :])

            def emit_body():
                for mc in range(NMACRO):
                    sl = slice(mc * BM, (mc + 1) * BM)
                    if mc == 0:
                        xh_t, c_t = xh0_t, c0_t
                    else:
                        xh_t = io.tile([P, KT, BM], F16, tag="xh")
                        nc.sync.dma_start(out=xh_t[:], in_=xhT_r[:, :, sl])
                        c_t = io.tile([P, ND, BM], F16, tag="c")
                        nc.sync.dma_start(out=c_t[:], in_=CT_r[:, :, sl])
                    g_t = work.tile([P, NJ, BM], F16, tag="gates")
                    cn_t = io.tile([P, ND, BM], F16, tag="cn")
                    hn_t = io.tile([P, ND, BM], F16, tag="hn")

                    for d in range(ND):
                        # z^T column blocks for dh-block d: i->d, f->4+d,
                        # o->8+d, g->12+d. Evacuate in order i, f, g, o so
                        # the o gate (needed last, for h_new) lands last and
                        # the per-block tail is minimal.
                        for jb, af in (
                            (d, AF.Sigmoid),
                            (ND + d, AF.Sigmoid),
                            (3 * ND + d, AF.Tanh),
                            (2 * ND + d, AF.Sigmoid),
                        ):
                            zp = psum.tile([P, BM], F32, tag="zp")
                            for kt in range(KT):
                                nc.tensor.matmul(
                                    zp[:],
                                    w_t[:, jb, kt, :],
                                    xh_t[:, kt, :],
                                    start=(kt == 0),
                                    stop=(kt == KT - 1),
                                )
                            nc.scalar.activation(
                                g_t[:, jb, :], zp[:], af,
                                bias=bias_t[:, jb:jb + 1],
                            )
                        fc = work.tile([P, BM], F16, tag="fc")
                        nc.vector.tensor_mul(fc[:], g_t[:, ND + d, :], c_t[:, d, :])
                        ig = work.tile([P, BM], F16, tag="ig")
                        nc.vector.tensor_mul(ig[:], g_t[:, d, :], g_t[:, 3 * ND + d, :])
                        nc.vector.tensor_add(cn_t[:, d, :], fc[:], ig[:])
                        nc.sync.dma_start(out=Cn_r[:, d, sl], in_=cn_t[:, d, :])
                        th = work.tile([P, BM], F16, tag="th")
                        nc.scalar.activation(th[:], cn_t[:, d, :], AF.Tanh)
                        nc.vector.tensor_mul(hn_t[:, d, :], g_t[:, 2 * ND + d, :], th[:])
                        nc.sync.dma_start(out=Hn_r[:, d, sl], in_=hn_t[:, d, :])

            if loop_n:
                with tc.For_i(0, loop_n, 1):
                    for _ in range(unroll):
                        emit_body()
            else:
                emit_body()
    fanout_multi_waits(nc)
    return nc


_NC = None


def _get_nc():
    global _NC
    if _NC is None:
        _NC = build_nc()
    return _NC


def make_in_maps(x, C, h, Wx, bx, Wh, bh):
    x = np.asarray(x, dtype=np.float32)
    C = np.asarray(C, dtype=np.float32)
    h = np.asarray(h, dtype=np.float32)
    W16 = np.concatenate(
        [np.asarray(Wx, np.float32), np.asarray(Wh, np.float32)], axis=0
    ).astype(np.float16)                                    # [1024, 2048]
    # swizzle to [p, jb, kt, jc]: W_pm[p, jb*1024 + kt*128 + jc]
    #   = W16[kt*128 + p, jb*128 + jc]
    W_pm = np.ascontiguousarray(
        W16.reshape(KT, P, NJ, P).transpose(1, 2, 0, 3)
    ).reshape(P, NJ * K)                                    # [128, 16384]
    bias = np.asarray(bx, np.float32) + np.asarray(bh, np.float32)
    biasT = np.ascontiguousarray(bias.reshape(NJ, P).T)     # [128, 16]
    in_maps = []
    for c in range(N_CORES):
        sl = slice(c * B_CORE, (c + 1) * B_CORE)
        xhT = np.empty((K, B_CORE), dtype=np.float16)
        xhT[:DH] = x[sl].T
        xhT[DH:] = h[sl].T
        CT = np.ascontiguousarray(C[sl].T.astype(np.float16))
        in_maps.append({"xhT": xhT, "CT": CT, "W": W_pm, "biasT": biasT})
    return in_maps


def kernel(x, C, h, Wx, bx, Wh, bh):
    nc = _get_nc()
    in_maps = make_in_maps(x, C, h, Wx, bx, Wh, bh)
    res = run_bass_kernel_spmd(nc, in_maps, list(range(N_CORES)))
    C_new = np.concatenate(
        [res.results[c]["CnT"].astype(np.float32).T for c in range(N_CORES)], axis=0
    )
    h_new = np.concatenate(
        [res.results[c]["HnT"].astype(np.float32).T for c in range(N_CORES)], axis=0
    )
    return (C_new, h_new)


# revision 19
# speedup vs baseline: 1.0254x; 1.0254x over previous
"""LSTMCell forward on 8 Trainium2 NeuronCores (Bass/Tile, SPMD data-parallel).

Strategy:
  - Shard the batch (32768) across 8 cores: 4096 rows each. The kernel is
    Tensor-engine bound: 2*4096*1024*2048 = 17.2 GFLOP/core at 78.6 TF/s
    (1 col/cycle @ 2.4 GHz for 16-bit operands) floors the body at ~220 us.
    fp8 DoubleRow (2x PE) was evaluated and rejected: e4m3 operand
    quantization gives 9.5e-2 rel err vs the 2e-2 tolerance (measured on the
    actual inputs); fp16 gives 8e-4.
  - All IO in fp16: halves HBM traffic vs fp32 (24 MB/core, ~67 us at
    358 GB/s — fully hidden under the PE) and doubles DVE throughput.
  - Transposed-z layout: compute z^T [2048 gate dims, batch] so the gate dim
    lands on PSUM partitions. Stationary operand = W 128x128 blocks, moving
    operand = xh^T batch columns (N=512, the PSUM-bank max, so LDWEIGHTS
    hides under the 512-cycle stream). The per-gate bias is then
    per-partition, so ACT fuses PSUM-evacuation + bias + sigmoid/tanh into
    ONE pass (no DVE bias add).
  - Per output block jb (16 of them: gates i,f,o,g x 4 dh-blocks): accumulate
    8 k-chunks into one PSUM bank [128, 512] (8-bank rotation keeps the PE
    several groups ahead of the ACT drain), ACT-evacuate to SBUF fp16.
  - jb order is gate-major per dh-block (i,f,o,g of block d together) so the
    elementwise epilogue for block d (C_new = f*C + i*g; h_new = o*tanh)
    runs on DVE while the PE streams block d+1; the epilogue tanh is
    deferred past the evacuations to keep the in-order ACT queue draining
    PSUM without stalls.
  - Batch tiled in macro-chunks of 512 columns, double-buffered DMA.
  - Prologue: macro-0 inputs and the first weight block are DMA'd ahead of
    the remaining 15 weight blocks (host pre-swizzles W so each block is a
    contiguous 4 KB/partition transfer), so the PE starts ~3.5 us in instead
    of waiting for the full 4 MB weight load.
"""
import sys
from contextlib import nullcontext

if "/opt/trn_rl_repo" not in sys.path:
    sys.path.insert(0, "/opt/trn_rl_repo")

import numpy as np
import concourse.bass as bass
import concourse.mybir as mybir
from concourse.tile import TileContext
from concourse.bass_utils import run_bass_kernel_spmd

F32 = mybir.dt.float32
F16 = mybir.dt.float16
AF = mybir.ActivationFunctionType

N_CORES = 8
P = 128
DH = 512
DH4 = 4 * DH            # 2048
K = 1024                # concat(x, h) contraction dim
KT = K // P             # 8 k-chunks
NJ = DH4 // P           # 16 output column blocks of z^T
ND = DH // P            # 4 dh blocks
B_FULL = 32768
B_CORE = B_FULL // N_CORES   # 4096
BM = 512                     # batch columns per macro-chunk (= PSUM bank)
NMACRO = B_CORE // BM        # 8


def fanout_multi_waits(nc):
    """This walrus build rejects >1 sync wait per instruction: fan extra
    waits out onto single-wait NoOps on the same (in-order) engine."""
    n = 0
    for f in nc.m.functions:
        for bb in f.blocks:
            new = []
            for inst in bb.instructions:
                si = inst.sync_info
                waits = list(si.on_wait) if si and si.on_wait else []
                if len(waits) > 1:
                    for w in waits[:-1]:
                        nop = mybir.InstNoOp(name=f"waitfan_{n}", ins=[], outs=[])
                        n += 1
                        nop.engine = inst.engine
                        nop.sync_info = mybir.SyncInfo(on_wait=[w], on_update=[])
                        new.append(nop)
                    si.on_wait = [waits[-1]]
                new.append(inst)
            bb.instructions = new
    return n


def build_nc(loop_n=None, unroll=1):
    """Build the per-core program. loop_n wraps the body in a device-side
    For_i repeat (timing probe; outputs unchanged since the body is
    idempotent). unroll emits that many body copies per loop iteration —
    the For_i back edge is a cross-engine sync point that idles the PE for
    several us, so fewer iterations -> less measurement artifact."""
    nc = bass.Bass()
    xhT = nc.dram_tensor("xhT", [K, B_CORE], F16, kind="ExternalInput")
    CT = nc.dram_tensor("CT", [DH, B_CORE], F16, kind="ExternalInput")
    # W host-swizzled to [128, jb, kt*128] so each jb block is a fully
    # contiguous 4 KB-per-partition DMA.
    W = nc.dram_tensor("W", [P, NJ * K], F16, kind="ExternalInput")
    biasT = nc.dram_tensor("biasT", [P, NJ], F32, kind="ExternalInput")
    CnT = nc.dram_tensor("CnT", [DH, B_CORE], F16, kind="ExternalOutput")
    HnT = nc.dram_tensor("HnT", [DH, B_CORE], F16, kind="ExternalOutput")

    xhT_r = xhT[:].rearrange("(kt p) b -> p kt b", p=P)     # [128, 8, 4096]
    W_r = W[:].rearrange("p (j kt c) -> p j kt c", j=NJ, kt=KT)  # [128,16,8,128]
    CT_r = CT[:].rearrange("(d p) b -> p d b", p=P)         # [128, 4, 4096]
    Cn_r = CnT[:].rearrange("(d p) b -> p d b", p=P)
    Hn_r = HnT[:].rearrange("(d p) b -> p d b", p=P)

    with TileContext(nc) as tc:
        with (
            tc.tile_pool(name="const", bufs=1) as const,
            tc.tile_pool(name="io", bufs=2) as io,
            tc.tile_pool(name="work", bufs=2) as work,
            tc.tile_pool(name="psum", bufs=4, space=bass.MemorySpace.PSUM) as psum,
        ):
            # Split the weight load per column-block (jb0 first) so the first
            # matmul group only waits on ~300 KB, not the whole 4 MB.
            w_t = const.tile([P, NJ, KT, P], F16)
            bias_t = const.tile([P, NJ], F32)
            # Macro-0 inputs first: the PE can start ~3.5 us in (xh0 + W jb0)
            # while the remaining W blocks stream in behind it.
            xh0_t = const.tile([P, KT, BM], F16)
            nc.sync.dma_start(out=xh0_t[:], in_=xhT_r[:, :, 0:BM])
            nc.sync.dma_start(out=w_t[:, 0, :, :], in_=W_r[:, 0, :, :])
            nc.sync.dma_start(out=bias_t[:], in_=biasT[:])
            c0_t = const.tile([P, ND, BM], F16)
            nc.sync.dma_start(out=c0_t[:], in_=CT_r[:, :, 0:BM])
            for jb in range(1, NJ):
                nc.sync.dma_start(out=w_t[:, jb, :, :], in_=W_r[:, jb, :, :])

            def emit_body():
                for mc in range(NMACRO):
                    sl = slice(mc * BM, (mc + 1) * BM)
                    if mc == 0:
                        xh_t, c_t = xh0_t, c0_t
                    else:
                        xh_t = io.tile([P, KT, BM], F16, tag="xh")
                        nc.sync.dma_start(out=xh_t[:], in_=xhT_r[:, :, sl])
                        c_t = io.tile([P, ND, BM], F16, tag="c")
                        nc.sync.dma_start(out=c_t[:], in_=CT_r[:, :, sl])
                    g_t = work.tile([P, NJ, BM], F16, tag="gates")
                    cn_t = io.tile([P, ND, BM], F16, tag="cn")
                    hn_t = io.tile([P, ND, BM], F16, tag="hn")

                    for d in range(ND):
                        # z^T column blocks for dh-block d: i->d, f->4+d,
                        # o->8+d, g->12+d. Gate order i, f, g, o so the o
                        # gate (needed last, for h_new) lands last and the
                        # per-block tail is minimal. Accumulation groups run
                        # as interleaved pairs on two PSUM chains so a
                        # group-leader matmul never gates on the previous
                        # group's evacuation (K18 micro-idle pathology).
                        for (jbA, afA), (jbB, afB) in (
                            ((d, AF.Sigmoid), (ND + d, AF.Sigmoid)),
                            ((3 * ND + d, AF.Tanh), (2 * ND + d, AF.Sigmoid)),
                        ):
                            za = psum.tile([P, BM], F32, tag="za")
                            zb = psum.tile([P, BM], F32, tag="zb")
                            for kt in range(KT):
                                nc.tensor.matmul(
                                    za[:], w_t[:, jbA, kt, :], xh_t[:, kt, :],
                                    start=(kt == 0), stop=(kt == KT - 1),
                                )
                                nc.tensor.matmul(
                                    zb[:], w_t[:, jbB, kt, :], xh_t[:, kt, :],
                                    start=(kt == 0), stop=(kt == KT - 1),
                                )
                            nc.scalar.activation(
                                g_t[:, jbA, :], za[:], afA,
                                bias=bias_t[:, jbA:jbA + 1],
                            )
                            nc.scalar.activation(
                                g_t[:, jbB, :], zb[:], afB,
                                bias=bias_t[:, jbB:jbB + 1],
                            )
                        fc = work.tile([P, BM], F16, tag="fc")
                        nc.vector.tensor_mul(fc[:], g_t[:, ND + d, :], c_t[:, d, :])
                        ig = work.tile([P, BM], F16, tag="ig")
                        nc.vector.tensor_mul(ig[:], g_t[:, d, :], g_t[:, 3 * ND + d, :])
                        nc.vector.tensor_add(cn_t[:, d, :], fc[:], ig[:])
                        nc.sync.dma_start(out=Cn_r[:, d, sl], in_=cn_t[:, d, :])
                        th = work.tile([P, BM], F16, tag="th")
                        nc.scalar.activation(th[:], cn_t[:, d, :], AF.Tanh)
                        nc.vector.tensor_mul(hn_t[:, d, :], g_t[:, 2 * ND + d, :], th[:])
                        nc.sync.dma_start(out=Hn_r[:, d, sl], in_=hn_t[:, d, :])

            if loop_n:
                with tc.For_i(0, loop_n, 1):
                    for _ in range(unroll):
                        emit_body()
            else:
                emit_body()
    fanout_multi_waits(nc)
    return nc


_NC = None


def _get_nc():
    global _NC
    if _NC is None:
        _NC = build_nc()
    return _NC


def make_in_maps(x, C, h, Wx, bx, Wh, bh):
    x = np.asarray(x, dtype=np.float32)
    C = np.asarray(C, dtype=np.float32)
    h = np.asarray(h, dtype=np.float32)
    W16 = np.concatenate(
        [np.asarray(Wx, np.float32), np.asarray(Wh, np.float32)], axis=0
    ).astype(np.float16)                                    # [1024, 2048]
    # swizzle to [p, jb, kt, jc]: W_pm[p, jb*1024 + kt*128 + jc]
    #   = W16[kt*128 + p, jb*128 + jc]
    W_pm = np.ascontiguousarray(
        W16.reshape(KT, P, NJ, P).transpose(1, 2, 0, 3)
    ).reshape(P, NJ * K)                                    # [128, 16384]
    bias = np.asarray(bx, np.float32) + np.asarray(bh, np.float32)
    biasT = np.ascontiguousarray(bias.reshape(NJ, P).T)     # [128, 16]
    in_maps = []
    for c in range(N_CORES):
        sl = slice(c * B_CORE, (c + 1) * B_CORE)
        xhT = np.empty((K, B_CORE), dtype=np.float16)
        xhT[:DH] = x[sl].T
        xhT[DH:] = h[sl].T
        CT = np.ascontiguousarray(C[sl].T.astype(np.float16))
        in_maps.append({"xhT": xhT, "CT": CT, "W": W_pm, "biasT": biasT})
    return in_maps


def kernel(x, C, h, Wx, bx, Wh, bh):
    nc = _get_nc()
    in_maps = make_in_maps(x, C, h, Wx, bx, Wh, bh)
    res = run_bass_kernel_spmd(nc, in_maps, list(range(N_CORES)))
    C_new = np.concatenate(
        [res.results[c]["CnT"].astype(np.float32).T for c in range(N_CORES)], axis=0
    )
    h_new = np.concatenate(
        [res.results[c]["HnT"].astype(np.float32).T for c in range(N_CORES)], axis=0
    )
    return (C_new, h_new)


# revision 20
# speedup vs baseline: 1.0534x; 1.0273x over previous
"""LSTMCell forward on 8 Trainium2 NeuronCores (Bass/Tile, SPMD data-parallel).

Strategy:
  - Shard the batch (32768) across 8 cores: 4096 rows each. The kernel is
    Tensor-engine bound: 2*4096*1024*2048 = 17.2 GFLOP/core at 78.6 TF/s
    (1 col/cycle @ 2.4 GHz for 16-bit operands) floors the body at ~220 us.
    fp8 DoubleRow (2x PE) was evaluated and rejected: e4m3 operand
    quantization gives 9.5e-2 rel err vs the 2e-2 tolerance (measured on the
    actual inputs); fp16 gives 8e-4.
  - All IO in fp16: halves HBM traffic vs fp32 (24 MB/core, ~67 us at
    358 GB/s — fully hidden under the PE) and doubles DVE throughput.
  - Transposed-z layout: compute z^T [2048 gate dims, batch] so the gate dim
    lands on PSUM partitions. Stationary operand = W 128x128 blocks, moving
    operand = xh^T batch columns (N=512, the PSUM-bank max, so LDWEIGHTS
    hides under the 512-cycle stream). The per-gate bias is then
    per-partition, so ACT fuses PSUM-evacuation + bias + sigmoid/tanh into
    ONE pass (no DVE bias add).
  - Per output block jb (16 of them: gates i,f,o,g x 4 dh-blocks): accumulate
    8 k-chunks into one PSUM bank [128, 512], ACT-evacuate to SBUF fp16.
    Accumulation groups run as interleaved PAIRS on two PSUM chains (2 tags x
    4 bufs = all 8 banks): a group-leader matmul never gates on the previous
    group's evacuation, so the PE queue has no micro-idles (the documented
    HAM clock-gate oscillation trigger).
  - jb order is gate-major per dh-block, evacuated i, f, g, o so the o gate
    (needed only for h_new = o*tanh(C_new)) lands last: the elementwise
    epilogue for block d (C_new = f*C + i*g on DVE, tanh on ACT) runs while
    the PE streams block d+1, and per-d output DMAs drain each block as soon
    as it is ready, leaving a ~1.6 us body tail.
  - Batch tiled in macro-chunks of 512 columns, double-buffered DMA.
  - Prologue: macro-0 inputs and the first weight block are DMA'd ahead of
    the remaining 15 weight blocks (host pre-swizzles W so each block is a
    contiguous 4 KB/partition transfer), so the PE starts ~3.5 us in instead
    of waiting for the full 4 MB weight load.
"""
import sys
from contextlib import nullcontext

if "/opt/trn_rl_repo" not in sys.path:
    sys.path.insert(0, "/opt/trn_rl_repo")

import numpy as np
import concourse.bass as bass
import concourse.mybir as mybir
from concourse.tile import TileContext
from concourse.bass_utils import run_bass_kernel_spmd

F32 = mybir.dt.float32
F16 = mybir.dt.float16
AF = mybir.ActivationFunctionType

N_CORES = 8
P = 128
DH = 512
DH4 = 4 * DH            # 2048
K = 1024                # concat(x, h) contraction dim
KT = K // P             # 8 k-chunks
NJ = DH4 // P           # 16 output column blocks of z^T
ND = DH // P            # 4 dh blocks
B_FULL = 32768
B_CORE = B_FULL // N_CORES   # 4096
BM = 512                     # batch columns per macro-chunk (= PSUM bank)
NMACRO = B_CORE // BM        # 8


def fanout_multi_waits(nc):
    """This walrus build rejects >1 sync wait per instruction: fan extra
    waits out onto single-wait NoOps on the same (in-order) engine."""
    n = 0
    for f in nc.m.functions:
        for bb in f.blocks:
            new = []
            for inst in bb.instructions:
                si = inst.sync_info
                waits = list(si.on_wait) if si and si.on_wait else []
                if len(waits) > 1:
                    for w in waits[:-1]:
                        nop = mybir.InstNoOp(name=f"waitfan_{n}", ins=[], outs=[])
                        n += 1
                        nop.engine = inst.engine
                        nop.sync_info = mybir.SyncInfo(on_wait=[w], on_update=[])
                        new.append(nop)
                    si.on_wait = [waits[-1]]
                new.append(inst)
            bb.instructions = new
    return n


def build_nc(loop_n=None, unroll=1):
    """Build the per-core program. loop_n wraps the body in a device-side
    For_i repeat (timing probe; outputs unchanged since the body is
    idempotent). unroll emits that many body copies per loop iteration —
    the For_i back edge is a cross-engine sync point that idles the PE for
    several us, so fewer iterations -> less measurement artifact."""
    nc = bass.Bass()
    xhT = nc.dram_tensor("xhT", [K, B_CORE], F16, kind="ExternalInput")
    CT = nc.dram_tensor("CT", [DH, B_CORE], F16, kind="ExternalInput")
    # W host-swizzled to [128, jb, kt*128] so each jb block is a fully
    # contiguous 4 KB-per-partition DMA.
    W = nc.dram_tensor("W", [P, NJ * K], F16, kind="ExternalInput")
    biasT = nc.dram_tensor("biasT", [P, NJ], F32, kind="ExternalInput")
    CnT = nc.dram_tensor("CnT", [DH, B_CORE], F16, kind="ExternalOutput")
    HnT = nc.dram_tensor("HnT", [DH, B_CORE], F16, kind="ExternalOutput")

    xhT_r = xhT[:].rearrange("(kt p) b -> p kt b", p=P)     # [128, 8, 4096]
    W_r = W[:].rearrange("p (j kt c) -> p j kt c", j=NJ, kt=KT)  # [128,16,8,128]
    CT_r = CT[:].rearrange("(d p) b -> p d b", p=P)         # [128, 4, 4096]
    Cn_r = CnT[:].rearrange("(d p) b -> p d b", p=P)
    Hn_r = HnT[:].rearrange("(d p) b -> p d b", p=P)

    with TileContext(nc) as tc:
        with (
            tc.tile_pool(name="const", bufs=1) as const,
            tc.tile_pool(name="io", bufs=2) as io,
            tc.tile_pool(name="work", bufs=2) as work,
            tc.tile_pool(name="psum", bufs=4, space=bass.MemorySpace.PSUM) as psum,
        ):
            # Split the weight load per column-block (jb0 first) so the first
            # matmul group only waits on ~300 KB, not the whole 4 MB.
            w_t = const.tile([P, NJ, KT, P], F16)
            bias_t = const.tile([P, NJ], F32)
            # Macro-0 inputs first: the PE can start ~3.5 us in (xh0 + W jb0)
            # while the remaining W blocks stream in behind it.
            xh0_t = const.tile([P, KT, BM], F16)
            nc.sync.dma_start(out=xh0_t[:], in_=xhT_r[:, :, 0:BM])
            nc.sync.dma_start(out=w_t[:, 0, :, :], in_=W_r[:, 0, :, :])
            nc.sync.dma_start(out=bias_t[:], in_=biasT[:])
            c0_t = const.tile([P, ND, BM], F16)
            nc.sync.dma_start(out=c0_t[:], in_=CT_r[:, :, 0:BM])
            for jb in range(1, NJ):
                nc.sync.dma_start(out=w_t[:, jb, :, :], in_=W_r[:, jb, :, :])

            def emit_body():
                for mc in range(NMACRO):
                    sl = slice(mc * BM, (mc + 1) * BM)
                    if mc == 0:
                        xh_t, c_t = xh0_t, c0_t
                    else:
                        xh_t = io.tile([P, KT, BM], F16, tag="xh")
                        nc.sync.dma_start(out=xh_t[:], in_=xhT_r[:, :, sl])
                        c_t = io.tile([P, ND, BM], F16, tag="c")
                        nc.sync.dma_start(out=c_t[:], in_=CT_r[:, :, sl])
                    g_t = work.tile([P, NJ, BM], F16, tag="gates")
                    cn_t = io.tile([P, ND, BM], F16, tag="cn")
                    hn_t = io.tile([P, ND, BM], F16, tag="hn")

                    for d in range(ND):
                        # z^T column blocks for dh-block d: i->d, f->4+d,
                        # o->8+d, g->12+d. Gate order i, f, g, o so the o
                        # gate (needed last, for h_new) lands last and the
                        # per-block tail is minimal. Accumulation groups run
                        # as interleaved pairs on two PSUM chains so a
                        # group-leader matmul never gates on the previous
                        # group's evacuation (K18 micro-idle pathology).
                        for (jbA, afA), (jbB, afB) in (
                            ((d, AF.Sigmoid), (ND + d, AF.Sigmoid)),
                            ((3 * ND + d, AF.Tanh), (2 * ND + d, AF.Sigmoid)),
                        ):
                            za = psum.tile([P, BM], F32, tag="za")
                            zb = psum.tile([P, BM], F32, tag="zb")
                            for kt in range(KT):
                                nc.tensor.matmul(
                                    za[:], w_t[:, jbA, kt, :], xh_t[:, kt, :],
                                    start=(kt == 0), stop=(kt == KT - 1),
                                )
                                nc.tensor.matmul(
                                    zb[:], w_t[:, jbB, kt, :], xh_t[:, kt, :],
                                    start=(kt == 0), stop=(kt == KT - 1),
                                )
                            nc.scalar.activation(
                                g_t[:, jbA, :], za[:], afA,
                                bias=bias_t[:, jbA:jbA + 1],
                            )
                            nc.scalar.activation(
                                g_t[:, jbB, :], zb[:], afB,
                                bias=bias_t[:, jbB:jbB + 1],
                            )
                        fc = work.tile([P, BM], F16, tag="fc")
                        nc.vector.tensor_mul(fc[:], g_t[:, ND + d, :], c_t[:, d, :])
                        ig = work.tile([P, BM], F16, tag="ig")
                        nc.vector.tensor_mul(ig[:], g_t[:, d, :], g_t[:, 3 * ND + d, :])
                        nc.vector.tensor_add(cn_t[:, d, :], fc[:], ig[:])
                        nc.sync.dma_start(out=Cn_r[:, d, sl], in_=cn_t[:, d, :])
                        th = work.tile([P, BM], F16, tag="th")
                        nc.scalar.activation(th[:], cn_t[:, d, :], AF.Tanh)
                        nc.vector.tensor_mul(hn_t[:, d, :], g_t[:, 2 * ND + d, :], th[:])
                        nc.sync.dma_start(out=Hn_r[:, d, sl], in_=hn_t[:, d, :])

            if loop_n:
                with tc.For_i(0, loop_n, 1):
                    for _ in range(unroll):
                        emit_body()
            else:
                emit_body()
    fanout_multi_waits(nc)
    return nc


_NC = None


def _get_nc():
    global _NC
    if _NC is None:
        _NC = build_nc()
    return _NC


def make_in_maps(x, C, h, Wx, bx, Wh, bh):
    x = np.asarray(x, dtype=np.float32)
    C = np.asarray(C, dtype=np.float32)
    h = np.asarray(h, dtype=np.float32)
    W16 = np.concatenate(
        [np.asarray(Wx, np.float32), np.asarray(Wh, np.float32)], axis=0
    ).astype(np.float16)                                    # [1024, 2048]
    # swizzle to [p, jb, kt, jc]: W_pm[p, jb*1024 + kt*128 + jc]
    #   = W16[kt*128 + p, jb*128 + jc]
    W_pm = np.ascontiguousarray(
        W16.reshape(KT, P, NJ, P).transpose(1, 2, 0, 3)
    ).reshape(P, NJ * K)                                    # [128, 16384]
    bias = np.asarray(bx, np.float32) + np.asarray(bh, np.float32)
    biasT = np.ascontiguousarray(bias.reshape(NJ, P).T)     # [128, 16]
    in_maps = []
    for c in range(N_CORES):
        sl = slice(c * B_CORE, (c + 1) * B_CORE)
        xhT = np.empty((K, B_CORE), dtype=np.float16)
        xhT[:DH] = x[sl].T
        xhT[DH:] = h[sl].T
        CT = np.ascontiguousarray(C[sl].T.astype(np.float16))
        in_maps.append({"xhT": xhT, "CT": CT, "W": W_pm, "biasT": biasT})
    return in_maps


def kernel(x, C, h, Wx, bx, Wh, bh):
    nc = _get_nc()
    in_maps = make_in_maps(x, C, h, Wx, bx, Wh, bh)
    res = run_bass_kernel_spmd(nc, in_maps, list(range(N_CORES)))
    C_new = np.concatenate(
        [res.results[c]["CnT"].astype(np.float32).T for c in range(N_CORES)], axis=0
    )
    h_new = np.concatenate(
        [res.results[c]["HnT"].astype(np.float32).T for c in range(N_CORES)], axis=0
    )
    return (C_new, h_new)
